# revision 6
# baseline (speedup 1.0000x reference)
"""Trainium2 Bass kernel for nn_CellEncoder (2-layer GraphSAGE, mean aggregation).

Strategy (8 NeuronCores, SPMD, node-partitioned):
  - Core c owns nodes [c*npc, (c+1)*npc).  Aggregation is linear, so the
    dense transform is applied FIRST: z = h @ W_l.T reduces gather width
    from in_dim (1000) to emb (128) floats per edge.
  - Per layer: each core computes z for its own nodes (node-major [n,128]
    rows), contributes two half-slabs to two AllGathers forming
    table_lo/table_hi (each NC*npc/2 rows < 32768 so rows are addressable
    by int16 dma_gather indices).
  - Edges are grouped by dst tile (128 dsts) on the host; each tile's edges
    are packed into chunks of 128 slots (lo-table chunks then hi-table
    chunks). dma_gather pulls the slot rows into SBUF; a one-hot matrix
    S[e,d] = (dstid[e]==d) is built on DVE (tensor_scalar is_equal, 2x
    mode) and the PE accumulates aggT[f,d] += G_chunk.T @ S_chunk in PSUM.
  - Epilogue runs feature-major (transposed): x = aggT*inv + rT + b;
    h = elu(x) = (max(x,0)-1) + exp(min(x,0)).  Feature-major h feeds the
    next layer's matmuls directly (no transposes anywhere).
  - Output written feature-major [128, NPAD]; host transposes/trims.

kernel(**inputs) takes FULL inputs, shards internally, runs one NEFF on
cores 0-7 via bass_utils.run_bass_kernel_spmd, returns the full output.
"""
import os
import sys

import numpy as np

for _p in ("/opt/trn_rl_repo", "/root/.axon_site/_ro/trn_rl_repo"):
    if os.path.isdir(_p) and _p not in sys.path:
        sys.path.append(_p)

import concourse.bass as bass
import concourse.bacc as bacc
import concourse.mybir as mybir
import concourse.tile as tile
from concourse import bass_utils

P = 128
F32 = mybir.dt.float32
AF = mybir.ActivationFunctionType
ALU = mybir.AluOpType


# ---------------------------------------------------------------------------
# host-side preprocessing
# ---------------------------------------------------------------------------

def build_meta(N, NC, dst, src, tiles_per_st):
    """Static chunk structure (shared across cores; max-over-core sizes) and
    per-core gather-index / dst-id slabs."""
    npc = N // NC
    half = npc // 2
    TPC = (npc + P - 1) // P
    NPAD = TPC * P
    NST = (TPC + tiles_per_st - 1) // tiles_per_st

    c = dst // npc
    d = (dst - c * npc).astype(np.int64)
    t = d // P
    did = d % P
    sc = src // npc
    sp = src - sc * npc
    tb = (sp >= half).astype(np.int64)
    row = sc * half + np.where(tb == 0, sp, sp - half)
    assert row.max() < 32768

    nlohi = np.zeros((NC, TPC, 2), np.int64)
    np.add.at(nlohi, (c, t, tb), 1)
    KL = np.maximum(1, (nlohi[:, :, 0].max(axis=0) + P - 1) // P)
    KH = ((nlohi[:, :, 1].max(axis=0) + P - 1) // P).astype(np.int64)

    Ktot = KL + KH
    chunk_base = np.concatenate([[0], np.cumsum(Ktot)])
    NCHUNK = int(chunk_base[-1])

    st_tiles = [list(range(s * tiles_per_st, min((s + 1) * tiles_per_st, TPC)))
                for s in range(NST)]
    GL = [int(sum(KL[tt] for tt in ts)) for ts in st_tiles]
    GH = [int(sum(KH[tt] for tt in ts)) for ts in st_tiles]

    idx_off_lo, idx_off_hi = [], []
    off = 0
    for s in range(NST):
        idx_off_lo.append(off); off += GL[s] * P // 16
        idx_off_hi.append(off); off += GH[s] * P // 16
    NIDX16 = off

    idx_slab = np.zeros((NC, P, NIDX16), np.int16)
    dstid_slab = np.full((NC, P, NCHUNK), -1.0, np.float32)
    cnt = np.zeros((NC, NPAD), np.int64)

    order = np.lexsort((tb, t, c))
    co, to, tbo = c[order], t[order], tb[order]
    rowo, dido, do_ = row[order], did[order], d[order]
    np.add.at(cnt, (co, do_), 1)

    key = (co * TPC + to) * 2 + tbo
    bounds = np.concatenate([[0], np.nonzero(np.diff(key))[0] + 1, [len(key)]])
    gval_lo = [np.zeros((NC, GL[s] * P), np.int16) for s in range(NST)]
    gval_hi = [np.zeros((NC, GH[s] * P), np.int16) for s in range(NST)]

    lo_base = np.zeros(TPC, np.int64)
    hi_base = np.zeros(TPC, np.int64)
    for s, ts in enumerate(st_tiles):
        accl = acch = 0
        for tt in ts:
            lo_base[tt] = accl; accl += KL[tt] * P
            hi_base[tt] = acch; acch += KH[tt] * P

    for bi in range(len(bounds) - 1):
        lo_, hi_ = int(bounds[bi]), int(bounds[bi + 1])
        if lo_ == hi_:
            continue
        cc, tt, bb = int(co[lo_]), int(to[lo_]), int(tbo[lo_])
        n = hi_ - lo_
        s = tt // tiles_per_st
        if bb == 0:
            base = int(lo_base[tt])
            gval_lo[s][cc, base:base + n] = rowo[lo_:hi_]
            ch0 = int(chunk_base[tt])
        else:
            base = int(hi_base[tt])
            gval_hi[s][cc, base:base + n] = rowo[lo_:hi_]
            ch0 = int(chunk_base[tt]) + int(KL[tt])
        # base is a multiple of P: slot partition (base+i)%P == i%P and
        # gather block base//P + i//P lines up with tile chunk ch0 + i//P.
        local = np.arange(n)
        dstid_slab[cc, local % P, ch0 + local // P] = dido[lo_:hi_]

    for s in range(NST):
        for cc in range(NC):
            for vals, o in ((gval_lo[s][cc], idx_off_lo[s]),
                            (gval_hi[s][cc], idx_off_hi[s])):
                n = len(vals)
                if n == 0:
                    continue
                w = vals.reshape(n // 16, 16).T
                idx_slab[cc, :, o:o + n // 16] = np.tile(w, (8, 1))

    inv = (1.0 / np.maximum(cnt, 1)).astype(np.float32)

    return dict(
        npc=npc, half=half, TPC=TPC, NPAD=NPAD, NST=NST, st_tiles=st_tiles,
        KL=[int(v) for v in KL], KH=[int(v) for v in KH],
        chunk_base=[int(v) for v in chunk_base], NCHUNK=NCHUNK,
        GL=GL, GH=GH, idx_off_lo=idx_off_lo, idx_off_hi=idx_off_hi,
        NIDX16=NIDX16, idx_slab=idx_slab, dstid_slab=dstid_slab, inv=inv,
    )


# ---------------------------------------------------------------------------
# device kernel builder
# ---------------------------------------------------------------------------

def build_kernel(meta, in_dim, NC):
    npc, half = meta["npc"], meta["half"]
    TPC, NPAD, NST = meta["TPC"], meta["NPAD"], meta["NST"]
    NCHUNK, NIDX16 = meta["NCHUNK"], meta["NIDX16"]
    KL, KH, chunk_base = meta["KL"], meta["KH"], meta["chunk_base"]
    GC = (in_dim + P - 1) // P
    GPAD = GC * P

    nc = bacc.Bacc("TRN2", target_bir_lowering=False, debug=False,
                   enable_asserts=False, num_devices=NC)

    x_d = nc.dram_tensor("x_pad", [GPAD, NPAD], F32, kind="ExternalInput").ap()
    w0l_d = nc.dram_tensor("W0lT", [GPAD, P], F32, kind="ExternalInput").ap()
    w0r_d = nc.dram_tensor("W0rT", [GPAD, P], F32, kind="ExternalInput").ap()
    w1l_d = nc.dram_tensor("W1lT", [P, P], F32, kind="ExternalInput").ap()
    w1r_d = nc.dram_tensor("W1rT", [P, P], F32, kind="ExternalInput").ap()
    b0_d = nc.dram_tensor("b0col", [P, 1], F32, kind="ExternalInput").ap()
    b1_d = nc.dram_tensor("b1col", [P, 1], F32, kind="ExternalInput").ap()
    iota_d = nc.dram_tensor("iota", [P, P], F32, kind="ExternalInput").ap()
    inv_d = nc.dram_tensor("invt", [P, NPAD], F32, kind="ExternalInput").ap()
    idx_d = nc.dram_tensor("idx16", [P, NIDX16], mybir.dt.int16,
                           kind="ExternalInput").ap()
    dst_d = nc.dram_tensor("dstid", [P, NCHUNK], F32, kind="ExternalInput").ap()
    out_d = nc.dram_tensor("outT", [P, NPAD], F32, kind="ExternalOutput").ap()

    with tile.TileContext(nc, num_cores=NC) as tc:
        with (
            tc.tile_pool(name="const", bufs=1) as cpool,
            tc.tile_pool(name="slab", bufs=1) as slab,
            tc.tile_pool(name="xp", bufs=2) as xpool,
            tc.tile_pool(name="gat", bufs=2) as gpool,
            tc.tile_pool(name="sp", bufs=4) as spool,
            tc.tile_pool(name="zp", bufs=3) as zpool,
            tc.tile_pool(name="ep", bufs=2) as epool,
            tc.tile_pool(name="pz", bufs=3, space="PSUM") as pz,
            tc.tile_pool(name="pr", bufs=2, space="PSUM") as pr,
            tc.tile_pool(name="pa", bufs=2, space="PSUM") as pa,
            tc.tile_pool(name="dram", bufs=1, space="DRAM") as dram,
        ):
            # ---- constants ----
            w0l_sb = cpool.tile([P, GC * P], F32)
            w0r_sb = cpool.tile([P, GC * P], F32)
            for gc in range(GC):
                nc.sync.dma_start(out=w0l_sb[:, gc * P:(gc + 1) * P],
                                  in_=w0l_d[gc * P:(gc + 1) * P, :])
                nc.sync.dma_start(out=w0r_sb[:, gc * P:(gc + 1) * P],
                                  in_=w0r_d[gc * P:(gc + 1) * P, :])
            w1l_sb = cpool.tile([P, P], F32)
            nc.sync.dma_start(out=w1l_sb[:], in_=w1l_d[:])
            w1r_sb = cpool.tile([P, P], F32)
            nc.sync.dma_start(out=w1r_sb[:], in_=w1r_d[:])
            b0_sb = cpool.tile([P, 1], F32)
            nc.sync.dma_start(out=b0_sb[:], in_=b0_d[:])
            b1_sb = cpool.tile([P, 1], F32)
            nc.sync.dma_start(out=b1_sb[:], in_=b1_d[:])
            iota_sb = cpool.tile([P, P], F32)
            nc.sync.dma_start(out=iota_sb[:], in_=iota_d[:])
            inv_sb = cpool.tile([P, NPAD], F32)
            nc.sync.dma_start(out=inv_sb[:], in_=inv_d[:])
            idx_sb = cpool.tile([P, NIDX16], mybir.dt.int16)
            nc.sync.dma_start(out=idx_sb[:], in_=idx_d[:])
            dst_sb = cpool.tile([P, NCHUNK], F32)
            nc.sync.dma_start(out=dst_sb[:], in_=dst_d[:])

            rb0_sb = slab.tile([P, NPAD], F32)
            rb1_sb = slab.tile([P, NPAD], F32)

            # ---- collective buffers ----
            def cc_pair(nm):
                i_lo = dram.tile([half, P], F32, name=f"cci_lo{nm}")
                i_hi = dram.tile([half, P], F32, name=f"cci_hi{nm}")
                o_lo = dram.tile([NC * half, P], F32, addr_space="Shared",
                                 name=f"cco_lo{nm}")
                o_hi = dram.tile([NC * half, P], F32, addr_space="Shared",
                                 name=f"cco_hi{nm}")
                return i_lo, i_hi, o_lo, o_hi

            cc0 = cc_pair("0")
            cc1 = cc_pair("1")
            rg = [list(range(NC))]

            def z_to_cc(z_sb, tt, cc):
                """DMA node-major z tile (rows tt*128..) into the lo/hi
                collective input slabs, splitting at `half` / clipping npc."""
                r0, r1 = tt * P, min(tt * P + P, npc)
                for lo_s, hi_s, tgt, base in (
                        (r0, min(r1, half), cc[0], 0),
                        (max(r0, half), r1, cc[1], half)):
                    if hi_s > lo_s:
                        nc.sync.dma_start(
                            out=tgt[lo_s - base:hi_s - base, :],
                            in_=z_sb[lo_s - r0:hi_s - r0, :])

            # ---- phase A: z0 (node-major) + rb0T (feature-major) ----
            for s, ts in enumerate(meta["st_tiles"]):
                w = len(ts) * P
                c0 = ts[0] * P
                xg = xpool.tile([P, GC * w], F32, tag="xg",
                                padded_shape=[P, GC * 512 if TPC >= 4 else GC * w])
                nc.sync.dma_start(
                    out=xg[:].rearrange("p (gc j) -> p gc j", gc=GC),
                    in_=x_d[:, c0:c0 + w].rearrange("(gc p) j -> p gc j", p=P))
                r0ps = pr.tile([P, w], F32, tag="rps", padded_shape=[P, 512])
                for gc in range(GC):
                    nc.tensor.matmul(out=r0ps[:],
                                     lhsT=w0r_sb[:, gc * P:(gc + 1) * P],
                                     rhs=xg[:, gc * w:(gc + 1) * w],
                                     start=(gc == 0), stop=(gc == GC - 1))
                nc.vector.tensor_scalar(out=rb0_sb[:, c0:c0 + w], in0=r0ps[:],
                                        scalar1=b0_sb[:, :1], scalar2=None,
                                        op0=ALU.add)
                for ti, tt in enumerate(ts):
                    z0ps = pz.tile([P, P], F32, tag="zps")
                    for gc in range(GC):
                        nc.tensor.matmul(
                            out=z0ps[:],
                            lhsT=xg[:, gc * w + ti * P:gc * w + (ti + 1) * P],
                            rhs=w0l_sb[:, gc * P:(gc + 1) * P],
                            start=(gc == 0), stop=(gc == GC - 1))
                    z0sb = zpool.tile([P, P], F32, tag="zsb")
                    nc.vector.tensor_copy(out=z0sb[:], in_=z0ps[:])
                    z_to_cc(z0sb, tt, cc0)

            nc.gpsimd.collective_compute(
                "AllGather", ALU.bypass, replica_groups=rg,
                ins=[cc0[0][:].opt()], outs=[cc0[2][:].opt()])
            nc.gpsimd.collective_compute(
                "AllGather", ALU.bypass, replica_groups=rg,
                ins=[cc0[1][:].opt()], outs=[cc0[3][:].opt()])

            # ---- aggregation + epilogue (shared for both layers) ----
            def aggregate(s, ts, tables, rb_slab, out_cb):
                w = len(ts) * P
                c0 = ts[0] * P
                GLs, GHs = meta["GL"][s], meta["GH"][s]
                gmax = max(max(meta["GL"]), max(meta["GH"]))
                glo = gpool.tile([P, max(GLs, 1) * P], F32, tag="glo",
                                 padded_shape=[P, gmax * P])
                ghi = gpool.tile([P, max(GHs, 1) * P], F32, tag="ghi",
                                 padded_shape=[P, gmax * P])
                # single-packet mode caps at 64 descs/engine = 1024 idxs
                if GLs:
                    nc.gpsimd.dma_gather(
                        out_ap=glo[:].rearrange("p (k e) -> p k e", e=P),
                        in_ap=tables[0][:],
                        idxs_ap=idx_sb[:, meta["idx_off_lo"][s]:
                                       meta["idx_off_lo"][s] + GLs * 8],
                        num_idxs=GLs * P, num_idxs_reg=GLs * P, elem_size=P,
                        single_packet=(GLs * P <= 1024))
                if GHs:
                    nc.gpsimd.dma_gather(
                        out_ap=ghi[:].rearrange("p (k e) -> p k e", e=P),
                        in_ap=tables[1][:],
                        idxs_ap=idx_sb[:, meta["idx_off_hi"][s]:
                                       meta["idx_off_hi"][s] + GHs * 8],
                        num_idxs=GHs * P, num_idxs_reg=GHs * P, elem_size=P,
                        single_packet=(GHs * P <= 1024))
                aggps = pa.tile([P, w], F32, tag="aggps", padded_shape=[P, 512])
                lo_blk = hi_blk = 0
                for ti, tt in enumerate(ts):
                    nch = KL[tt] + KH[tt]
                    for j in range(nch):
                        cg = chunk_base[tt] + j
                        if j < KL[tt]:
                            g_ap = glo[:, (lo_blk + j) * P:(lo_blk + j + 1) * P]
                        else:
                            jj = j - KL[tt]
                            g_ap = ghi[:, (hi_blk + jj) * P:(hi_blk + jj + 1) * P]
                        s_sb = spool.tile([P, P], F32, tag="ssb")
                        nc.vector.tensor_scalar(out=s_sb[:], in0=iota_sb[:],
                                                scalar1=dst_sb[:, cg:cg + 1],
                                                scalar2=None, op0=ALU.is_equal)
                        nc.tensor.matmul(out=aggps[:, ti * P:(ti + 1) * P],
                                         lhsT=g_ap, rhs=s_sb[:],
                                         start=(j == 0), stop=(j == nch - 1))
                    lo_blk += KL[tt]
                    hi_blk += KH[tt]
                # epilogue: x = aggT*inv + rb ; h = (max(x,0)-1) + exp(min(x,0))
                x2 = epool.tile([P, w], F32, tag="x2", padded_shape=[P, 512])
                nc.vector.tensor_tensor(
                    out=x2[:], in0=aggps[:],
                    in1=inv_sb[:, c0:c0 + w], op=ALU.mult)
                x3 = epool.tile([P, w], F32, tag="x3", padded_shape=[P, 512])
                nc.vector.tensor_tensor(out=x3[:], in0=x2[:],
                                        in1=rb_slab[:, c0:c0 + w], op=ALU.add)
                xm = epool.tile([P, w], F32, tag="xm", padded_shape=[P, 512])
                nc.vector.tensor_scalar(out=xm[:], in0=x3[:], scalar1=0.0,
                                        scalar2=-1.0, op0=ALU.max, op1=ALU.add)
                xc = epool.tile([P, w], F32, tag="xc", padded_shape=[P, 512])
                nc.vector.tensor_scalar(out=xc[:], in0=x3[:], scalar1=0.0,
                                        scalar2=None, op0=ALU.min)
                xe = epool.tile([P, w], F32, tag="xe", padded_shape=[P, 512])
                nc.scalar.activation(out=xe[:], in_=xc[:], func=AF.Exp)
                h = epool.tile([P, w], F32, tag="h", padded_shape=[P, 512])
                nc.vector.tensor_add(out=h[:], in0=xm[:], in1=xe[:])
                out_cb(s, ts, w, c0, h)

            # ---- phase B+C: layer-0 aggregate -> h1T -> z1/rb1T ----
            def l0_out(s, ts, w, c0, h):
                for ti, tt in enumerate(ts):
                    z1ps = pz.tile([P, P], F32, tag="zps")
                    nc.tensor.matmul(out=z1ps[:],
                                     lhsT=h[:, ti * P:(ti + 1) * P],
                                     rhs=w1l_sb[:], start=True, stop=True)
                    z1sb = zpool.tile([P, P], F32, tag="zsb")
                    nc.vector.tensor_copy(out=z1sb[:], in_=z1ps[:])
                    z_to_cc(z1sb, tt, cc1)
                r1ps = pr.tile([P, w], F32, tag="rps", padded_shape=[P, 512])
                nc.tensor.matmul(out=r1ps[:], lhsT=w1r_sb[:], rhs=h[:],
                                 start=True, stop=True)
                nc.vector.tensor_scalar(out=rb1_sb[:, c0:c0 + w], in0=r1ps[:],
                                        scalar1=b1_sb[:, :1], scalar2=None,
                                        op0=ALU.add)

            for s, ts in enumerate(meta["st_tiles"]):
                aggregate(s, ts, (cc0[2], cc0[3]), rb0_sb, l0_out)

            nc.gpsimd.collective_compute(
                "AllGather", ALU.bypass, replica_groups=rg,
                ins=[cc1[0][:].opt()], outs=[cc1[2][:].opt()])
            nc.gpsimd.collective_compute(
                "AllGather", ALU.bypass, replica_groups=rg,
                ins=[cc1[1][:].opt()], outs=[cc1[3][:].opt()])

            # ---- phase D: layer-1 aggregate -> output ----
            def l1_out(s, ts, w, c0, h):
                nc.sync.dma_start(out=out_d[:, c0:c0 + w], in_=h[:])

            for s, ts in enumerate(meta["st_tiles"]):
                aggregate(s, ts, (cc1[2], cc1[3]), rb1_sb, l1_out)

    nc.compile()
    return nc


# ---------------------------------------------------------------------------
# entry point
# ---------------------------------------------------------------------------

def _prepare(x, knn_edge_index, W_l0, b_l0, W_r0, W_l1, b_l1, W_r1,
             NC=8, tiles_per_st=2):
    x = np.asarray(x, np.float32)
    e = np.asarray(knn_edge_index)
    in_dim, N = x.shape
    src, dst = e[0].astype(np.int64), e[1].astype(np.int64)
    meta = build_meta(N, NC, dst, src, tiles_per_st)
    npc, NPAD = meta["npc"], meta["NPAD"]
    GC = (in_dim + P - 1) // P
    GPAD = GC * P

    w0l = np.zeros((GPAD, P), np.float32); w0l[:in_dim] = np.asarray(W_l0).T
    w0r = np.zeros((GPAD, P), np.float32); w0r[:in_dim] = np.asarray(W_r0).T
    shared = {
        "W0lT": w0l, "W0rT": w0r,
        "W1lT": np.ascontiguousarray(np.asarray(W_l1, np.float32).T),
        "W1rT": np.ascontiguousarray(np.asarray(W_r1, np.float32).T),
        "b0col": np.asarray(b_l0, np.float32).reshape(P, 1),
        "b1col": np.asarray(b_l1, np.float32).reshape(P, 1),
        "iota": np.broadcast_to(np.arange(P, dtype=np.float32), (P, P)).copy(),
    }
    in_maps = []
    for c in range(NC):
        xp = np.zeros((GPAD, NPAD), np.float32)
        xp[:in_dim, :npc] = x[:, c * npc:(c + 1) * npc]
        m = dict(shared)
        m["x_pad"] = xp
        m["invt"] = np.ascontiguousarray(
            np.broadcast_to(meta["inv"][c], (P, NPAD)))
        m["idx16"] = np.ascontiguousarray(meta["idx_slab"][c])
        m["dstid"] = np.ascontiguousarray(meta["dstid_slab"][c])
        in_maps.append(m)
    return meta, in_dim, in_maps


def run(inputs, NC=8, tiles_per_st=2, trace=False, **run_kwargs):
    meta, in_dim, in_maps = _prepare(**inputs, NC=NC, tiles_per_st=tiles_per_st)
    nc = build_kernel(meta, in_dim, NC)
    res = bass_utils.run_bass_kernel_spmd(
        nc, in_maps, core_ids=list(range(NC)), trace=trace, **run_kwargs)
    npc = meta["npc"]
    out = np.concatenate(
        [res.results[c]["outT"][:, :npc].T for c in range(NC)], axis=0)
    return np.ascontiguousarray(out), res


def kernel(**inputs) -> np.ndarray:
    out, _ = run(inputs)
    return out


# revision 7
# speedup vs baseline: 1.3297x; 1.3297x over previous
"""Trainium2 Bass kernel for nn_CellEncoder (2-layer GraphSAGE, mean aggregation).

Strategy (8 NeuronCores, SPMD, node-partitioned):
  - Core c owns nodes [c*npc, (c+1)*npc).  Aggregation is linear, so the
    dense transform is applied FIRST: z = h @ W_l.T reduces gather width
    from in_dim (1000) to emb (128) floats per edge.
  - Per layer: each core computes z for its own nodes (node-major [n,128]
    rows), contributes two half-slabs to two AllGathers forming
    table_lo/table_hi (each NC*npc/2 rows < 32768 so rows are addressable
    by int16 dma_gather indices).
  - Edges are grouped by dst tile (128 dsts) on the host; each tile's edges
    are packed into chunks of 128 slots (lo-table chunks then hi-table
    chunks). dma_gather pulls the slot rows into SBUF; a one-hot matrix
    S[e,d] = (dstid[e]==d) is built on DVE (tensor_scalar is_equal, 2x
    mode) and the PE accumulates aggT[f,d] += G_chunk.T @ S_chunk in PSUM.
  - Epilogue runs feature-major (transposed): x = aggT*inv + rT + b;
    h = elu(x) = (max(x,0)-1) + exp(min(x,0)).  Feature-major h feeds the
    next layer's matmuls directly (no transposes anywhere).
  - Output written feature-major [128, NPAD]; host transposes/trims.

kernel(**inputs) takes FULL inputs, shards internally, runs one NEFF on
cores 0-7 via bass_utils.run_bass_kernel_spmd, returns the full output.
"""
import os
import sys

import numpy as np

for _p in ("/opt/trn_rl_repo", "/root/.axon_site/_ro/trn_rl_repo"):
    if os.path.isdir(_p) and _p not in sys.path:
        sys.path.append(_p)

import concourse.bass as bass
import concourse.bacc as bacc
import concourse.mybir as mybir
import concourse.tile as tile
from concourse import bass_utils

P = 128
F32 = mybir.dt.float32
AF = mybir.ActivationFunctionType
ALU = mybir.AluOpType


# ---------------------------------------------------------------------------
# host-side preprocessing
# ---------------------------------------------------------------------------

def build_meta(N, NC, dst, src, tiles_per_st):
    """Static chunk structure (shared across cores; max-over-core sizes) and
    per-core gather-index / dst-id slabs."""
    npc = N // NC
    half = npc // 2
    TPC = (npc + P - 1) // P
    NPAD = TPC * P
    NST = (TPC + tiles_per_st - 1) // tiles_per_st

    c = dst // npc
    d = (dst - c * npc).astype(np.int64)
    t = d // P
    did = d % P
    sc = src // npc
    sp = src - sc * npc
    tb = (sp >= half).astype(np.int64)
    row = sc * half + np.where(tb == 0, sp, sp - half)
    assert row.max() < 32768

    nlohi = np.zeros((NC, TPC, 2), np.int64)
    np.add.at(nlohi, (c, t, tb), 1)
    KL = np.maximum(1, (nlohi[:, :, 0].max(axis=0) + P - 1) // P)
    KH = ((nlohi[:, :, 1].max(axis=0) + P - 1) // P).astype(np.int64)

    Ktot = KL + KH
    chunk_base = np.concatenate([[0], np.cumsum(Ktot)])
    NCHUNK = int(chunk_base[-1])

    st_tiles = [list(range(s * tiles_per_st, min((s + 1) * tiles_per_st, TPC)))
                for s in range(NST)]
    GL = [int(sum(KL[tt] for tt in ts)) for ts in st_tiles]
    GH = [int(sum(KH[tt] for tt in ts)) for ts in st_tiles]

    idx_off_lo, idx_off_hi = [], []
    off = 0
    for s in range(NST):
        idx_off_lo.append(off); off += GL[s] * P // 16
        idx_off_hi.append(off); off += GH[s] * P // 16
    NIDX16 = off

    idx_slab = np.zeros((NC, P, NIDX16), np.int16)
    dstid_slab = np.full((NC, P, NCHUNK), -1.0, np.float32)
    cnt = np.zeros((NC, NPAD), np.int64)

    order = np.lexsort((tb, t, c))
    co, to, tbo = c[order], t[order], tb[order]
    rowo, dido, do_ = row[order], did[order], d[order]
    np.add.at(cnt, (co, do_), 1)

    key = (co * TPC + to) * 2 + tbo
    bounds = np.concatenate([[0], np.nonzero(np.diff(key))[0] + 1, [len(key)]])
    gval_lo = [np.zeros((NC, GL[s] * P), np.int16) for s in range(NST)]
    gval_hi = [np.zeros((NC, GH[s] * P), np.int16) for s in range(NST)]

    lo_base = np.zeros(TPC, np.int64)
    hi_base = np.zeros(TPC, np.int64)
    for s, ts in enumerate(st_tiles):
        accl = acch = 0
        for tt in ts:
            lo_base[tt] = accl; accl += KL[tt] * P
            hi_base[tt] = acch; acch += KH[tt] * P

    for bi in range(len(bounds) - 1):
        lo_, hi_ = int(bounds[bi]), int(bounds[bi + 1])
        if lo_ == hi_:
            continue
        cc, tt, bb = int(co[lo_]), int(to[lo_]), int(tbo[lo_])
        n = hi_ - lo_
        s = tt // tiles_per_st
        if bb == 0:
            base = int(lo_base[tt])
            gval_lo[s][cc, base:base + n] = rowo[lo_:hi_]
            ch0 = int(chunk_base[tt])
        else:
            base = int(hi_base[tt])
            gval_hi[s][cc, base:base + n] = rowo[lo_:hi_]
            ch0 = int(chunk_base[tt]) + int(KL[tt])
        # base is a multiple of P: slot partition (base+i)%P == i%P and
        # gather block base//P + i//P lines up with tile chunk ch0 + i//P.
        local = np.arange(n)
        dstid_slab[cc, local % P, ch0 + local // P] = dido[lo_:hi_]

    for s in range(NST):
        for cc in range(NC):
            for vals, o in ((gval_lo[s][cc], idx_off_lo[s]),
                            (gval_hi[s][cc], idx_off_hi[s])):
                n = len(vals)
                if n == 0:
                    continue
                w = vals.reshape(n // 16, 16).T
                idx_slab[cc, :, o:o + n // 16] = np.tile(w, (8, 1))

    inv = (1.0 / np.maximum(cnt, 1)).astype(np.float32)

    return dict(
        npc=npc, half=half, TPC=TPC, NPAD=NPAD, NST=NST, st_tiles=st_tiles,
        KL=[int(v) for v in KL], KH=[int(v) for v in KH],
        chunk_base=[int(v) for v in chunk_base], NCHUNK=NCHUNK,
        GL=GL, GH=GH, idx_off_lo=idx_off_lo, idx_off_hi=idx_off_hi,
        NIDX16=NIDX16, idx_slab=idx_slab, dstid_slab=dstid_slab, inv=inv,
    )


# ---------------------------------------------------------------------------
# device kernel builder
# ---------------------------------------------------------------------------

def build_kernel(meta, in_dim, NC):
    npc, half = meta["npc"], meta["half"]
    TPC, NPAD, NST = meta["TPC"], meta["NPAD"], meta["NST"]
    NCHUNK, NIDX16 = meta["NCHUNK"], meta["NIDX16"]
    KL, KH, chunk_base = meta["KL"], meta["KH"], meta["chunk_base"]
    GC = (in_dim + P - 1) // P
    GPAD = GC * P

    nc = bacc.Bacc("TRN2", target_bir_lowering=False, debug=False,
                   enable_asserts=False, num_devices=NC)

    x_d = nc.dram_tensor("x_pad", [GPAD, NPAD], F32, kind="ExternalInput").ap()
    w0l_d = nc.dram_tensor("W0lT", [GPAD, P], F32, kind="ExternalInput").ap()
    w0r_d = nc.dram_tensor("W0rT", [GPAD, P], F32, kind="ExternalInput").ap()
    w1l_d = nc.dram_tensor("W1lT", [P, P], F32, kind="ExternalInput").ap()
    w1r_d = nc.dram_tensor("W1rT", [P, P], F32, kind="ExternalInput").ap()
    b0_d = nc.dram_tensor("b0col", [P, 1], F32, kind="ExternalInput").ap()
    b1_d = nc.dram_tensor("b1col", [P, 1], F32, kind="ExternalInput").ap()
    iota_d = nc.dram_tensor("iota", [P, P], F32, kind="ExternalInput").ap()
    inv_d = nc.dram_tensor("invt", [P, NPAD], F32, kind="ExternalInput").ap()
    idx_d = nc.dram_tensor("idx16", [P, NIDX16], mybir.dt.int16,
                           kind="ExternalInput").ap()
    dst_d = nc.dram_tensor("dstid", [P, NCHUNK], F32, kind="ExternalInput").ap()
    out_d = nc.dram_tensor("outT", [P, NPAD], F32, kind="ExternalOutput").ap()

    with tile.TileContext(nc, num_cores=NC) as tc:
        with (
            tc.tile_pool(name="const", bufs=1) as cpool,
            tc.tile_pool(name="slab", bufs=1) as slab,
            tc.tile_pool(name="xp", bufs=2) as xpool,
            tc.tile_pool(name="gat", bufs=2) as gpool,
            tc.tile_pool(name="sp", bufs=4) as spool,
            tc.tile_pool(name="zp", bufs=3) as zpool,
            tc.tile_pool(name="ep", bufs=2) as epool,
            tc.tile_pool(name="pz", bufs=3, space="PSUM") as pz,
            tc.tile_pool(name="pr", bufs=2, space="PSUM") as pr,
            tc.tile_pool(name="pa", bufs=2, space="PSUM") as pa,
            tc.tile_pool(name="dram", bufs=1, space="DRAM") as dram,
        ):
            # ---- constants ----
            w0l_sb = cpool.tile([P, GC * P], F32)
            w0r_sb = cpool.tile([P, GC * P], F32)
            for gc in range(GC):
                nc.sync.dma_start(out=w0l_sb[:, gc * P:(gc + 1) * P],
                                  in_=w0l_d[gc * P:(gc + 1) * P, :])
                nc.sync.dma_start(out=w0r_sb[:, gc * P:(gc + 1) * P],
                                  in_=w0r_d[gc * P:(gc + 1) * P, :])
            w1l_sb = cpool.tile([P, P], F32)
            nc.sync.dma_start(out=w1l_sb[:], in_=w1l_d[:])
            w1r_sb = cpool.tile([P, P], F32)
            nc.sync.dma_start(out=w1r_sb[:], in_=w1r_d[:])
            b0_sb = cpool.tile([P, 1], F32)
            nc.sync.dma_start(out=b0_sb[:], in_=b0_d[:])
            b1_sb = cpool.tile([P, 1], F32)
            nc.sync.dma_start(out=b1_sb[:], in_=b1_d[:])
            iota_sb = cpool.tile([P, P], F32)
            nc.sync.dma_start(out=iota_sb[:], in_=iota_d[:])
            inv_sb = cpool.tile([P, NPAD], F32)
            nc.sync.dma_start(out=inv_sb[:], in_=inv_d[:])
            idx_sb = cpool.tile([P, NIDX16], mybir.dt.int16)
            nc.sync.dma_start(out=idx_sb[:], in_=idx_d[:])
            dst_sb = cpool.tile([P, NCHUNK], F32)
            nc.sync.dma_start(out=dst_sb[:], in_=dst_d[:])

            rb0_sb = slab.tile([P, NPAD], F32)
            rb1_sb = slab.tile([P, NPAD], F32)

            # ---- collective buffers ----
            def cc_pair(nm):
                i_lo = dram.tile([half, P], F32, name=f"cci_lo{nm}")
                i_hi = dram.tile([half, P], F32, name=f"cci_hi{nm}")
                o_lo = dram.tile([NC * half, P], F32, addr_space="Shared",
                                 name=f"cco_lo{nm}")
                o_hi = dram.tile([NC * half, P], F32, addr_space="Shared",
                                 name=f"cco_hi{nm}")
                return i_lo, i_hi, o_lo, o_hi

            cc0 = cc_pair("0")
            cc1 = cc_pair("1")
            rg = [list(range(NC))]

            def z_to_cc(z_sb, tt, cc):
                """DMA node-major z tile (rows tt*128..) into the lo/hi
                collective input slabs, splitting at `half` / clipping npc."""
                r0, r1 = tt * P, min(tt * P + P, npc)
                for lo_s, hi_s, tgt, base in (
                        (r0, min(r1, half), cc[0], 0),
                        (max(r0, half), r1, cc[1], half)):
                    if hi_s > lo_s:
                        nc.sync.dma_start(
                            out=tgt[lo_s - base:hi_s - base, :],
                            in_=z_sb[lo_s - r0:hi_s - r0, :])

            # ---- phase A: z0 (node-major) + rb0T (feature-major) ----
            for s, ts in enumerate(meta["st_tiles"]):
                w = len(ts) * P
                c0 = ts[0] * P
                xg = xpool.tile([P, GC * w], F32, tag="xg",
                                padded_shape=[P, GC * 512 if TPC >= 4 else GC * w])
                nc.sync.dma_start(
                    out=xg[:].rearrange("p (gc j) -> p gc j", gc=GC),
                    in_=x_d[:, c0:c0 + w].rearrange("(gc p) j -> p gc j", p=P))
                r0ps = pr.tile([P, w], F32, tag="rps", padded_shape=[P, 512])
                for gc in range(GC):
                    nc.tensor.matmul(out=r0ps[:],
                                     lhsT=w0r_sb[:, gc * P:(gc + 1) * P],
                                     rhs=xg[:, gc * w:(gc + 1) * w],
                                     start=(gc == 0), stop=(gc == GC - 1))
                nc.vector.tensor_scalar(out=rb0_sb[:, c0:c0 + w], in0=r0ps[:],
                                        scalar1=b0_sb[:, :1], scalar2=None,
                                        op0=ALU.add)
                for ti, tt in enumerate(ts):
                    z0ps = pz.tile([P, P], F32, tag="zps")
                    for gc in range(GC):
                        nc.tensor.matmul(
                            out=z0ps[:],
                            lhsT=xg[:, gc * w + ti * P:gc * w + (ti + 1) * P],
                            rhs=w0l_sb[:, gc * P:(gc + 1) * P],
                            start=(gc == 0), stop=(gc == GC - 1))
                    z0sb = zpool.tile([P, P], F32, tag="zsb")
                    nc.vector.tensor_copy(out=z0sb[:], in_=z0ps[:])
                    z_to_cc(z0sb, tt, cc0)

            nc.gpsimd.collective_compute(
                "AllGather", ALU.bypass, replica_groups=rg,
                ins=[cc0[0][:].opt()], outs=[cc0[2][:].opt()])
            nc.gpsimd.collective_compute(
                "AllGather", ALU.bypass, replica_groups=rg,
                ins=[cc0[1][:].opt()], outs=[cc0[3][:].opt()])

            # ---- aggregation + epilogue (shared for both layers) ----
            def aggregate(s, ts, tables, rb_slab, out_cb):
                w = len(ts) * P
                c0 = ts[0] * P
                GLs, GHs = meta["GL"][s], meta["GH"][s]
                gmax = max(max(meta["GL"]), max(meta["GH"]))
                glo = gpool.tile([P, max(GLs, 1) * P], F32, tag="glo",
                                 padded_shape=[P, gmax * P])
                ghi = gpool.tile([P, max(GHs, 1) * P], F32, tag="ghi",
                                 padded_shape=[P, gmax * P])
                # single-packet mode caps at 64 descs/engine = 1024 idxs
                if GLs:
                    nc.gpsimd.dma_gather(
                        out_ap=glo[:].rearrange("p (k e) -> p k e", e=P),
                        in_ap=tables[0][:],
                        idxs_ap=idx_sb[:, meta["idx_off_lo"][s]:
                                       meta["idx_off_lo"][s] + GLs * 8],
                        num_idxs=GLs * P, num_idxs_reg=GLs * P, elem_size=P,
                        single_packet=(GLs * P <= 1024))
                if GHs:
                    nc.gpsimd.dma_gather(
                        out_ap=ghi[:].rearrange("p (k e) -> p k e", e=P),
                        in_ap=tables[1][:],
                        idxs_ap=idx_sb[:, meta["idx_off_hi"][s]:
                                       meta["idx_off_hi"][s] + GHs * 8],
                        num_idxs=GHs * P, num_idxs_reg=GHs * P, elem_size=P,
                        single_packet=(GHs * P <= 1024))
                aggps = pa.tile([P, w], F32, tag="aggps", padded_shape=[P, 512])
                lo_blk = hi_blk = 0
                for ti, tt in enumerate(ts):
                    nch = KL[tt] + KH[tt]
                    for j in range(nch):
                        cg = chunk_base[tt] + j
                        if j < KL[tt]:
                            g_ap = glo[:, (lo_blk + j) * P:(lo_blk + j + 1) * P]
                        else:
                            jj = j - KL[tt]
                            g_ap = ghi[:, (hi_blk + jj) * P:(hi_blk + jj + 1) * P]
                        s_sb = spool.tile([P, P], F32, tag="ssb")
                        # TT with free-broadcast dstid: per-partition-scalar
                        # tensor_scalar measured ~1.2us/chunk on HW; TT is 1x
                        nc.vector.tensor_tensor(
                            out=s_sb[:],
                            in0=dst_sb[:, cg:cg + 1].to_broadcast([P, P]),
                            in1=iota_sb[:], op=ALU.is_equal)
                        nc.tensor.matmul(out=aggps[:, ti * P:(ti + 1) * P],
                                         lhsT=g_ap, rhs=s_sb[:],
                                         start=(j == 0), stop=(j == nch - 1))
                    lo_blk += KL[tt]
                    hi_blk += KH[tt]
                # epilogue: x = aggT*inv + rb ; h = (max(x,0)-1) + exp(min(x,0))
                x2 = epool.tile([P, w], F32, tag="x2", padded_shape=[P, 512])
                nc.vector.tensor_tensor(
                    out=x2[:], in0=aggps[:],
                    in1=inv_sb[:, c0:c0 + w], op=ALU.mult)
                x3 = epool.tile([P, w], F32, tag="x3", padded_shape=[P, 512])
                nc.vector.tensor_tensor(out=x3[:], in0=x2[:],
                                        in1=rb_slab[:, c0:c0 + w], op=ALU.add)
                xm = epool.tile([P, w], F32, tag="xm", padded_shape=[P, 512])
                nc.vector.tensor_scalar(out=xm[:], in0=x3[:], scalar1=0.0,
                                        scalar2=-1.0, op0=ALU.max, op1=ALU.add)
                xc = epool.tile([P, w], F32, tag="xc", padded_shape=[P, 512])
                nc.vector.tensor_scalar(out=xc[:], in0=x3[:], scalar1=0.0,
                                        scalar2=None, op0=ALU.min)
                xe = epool.tile([P, w], F32, tag="xe", padded_shape=[P, 512])
                nc.scalar.activation(out=xe[:], in_=xc[:], func=AF.Exp)
                h = epool.tile([P, w], F32, tag="h", padded_shape=[P, 512])
                nc.vector.tensor_add(out=h[:], in0=xm[:], in1=xe[:])
                out_cb(s, ts, w, c0, h)

            # ---- phase B+C: layer-0 aggregate -> h1T -> z1/rb1T ----
            def l0_out(s, ts, w, c0, h):
                for ti, tt in enumerate(ts):
                    z1ps = pz.tile([P, P], F32, tag="zps")
                    nc.tensor.matmul(out=z1ps[:],
                                     lhsT=h[:, ti * P:(ti + 1) * P],
                                     rhs=w1l_sb[:], start=True, stop=True)
                    z1sb = zpool.tile([P, P], F32, tag="zsb")
                    nc.vector.tensor_copy(out=z1sb[:], in_=z1ps[:])
                    z_to_cc(z1sb, tt, cc1)
                r1ps = pr.tile([P, w], F32, tag="rps", padded_shape=[P, 512])
                nc.tensor.matmul(out=r1ps[:], lhsT=w1r_sb[:], rhs=h[:],
                                 start=True, stop=True)
                nc.vector.tensor_scalar(out=rb1_sb[:, c0:c0 + w], in0=r1ps[:],
                                        scalar1=b1_sb[:, :1], scalar2=None,
                                        op0=ALU.add)

            for s, ts in enumerate(meta["st_tiles"]):
                aggregate(s, ts, (cc0[2], cc0[3]), rb0_sb, l0_out)

            nc.gpsimd.collective_compute(
                "AllGather", ALU.bypass, replica_groups=rg,
                ins=[cc1[0][:].opt()], outs=[cc1[2][:].opt()])
            nc.gpsimd.collective_compute(
                "AllGather", ALU.bypass, replica_groups=rg,
                ins=[cc1[1][:].opt()], outs=[cc1[3][:].opt()])

            # ---- phase D: layer-1 aggregate -> output ----
            def l1_out(s, ts, w, c0, h):
                nc.sync.dma_start(out=out_d[:, c0:c0 + w], in_=h[:])

            for s, ts in enumerate(meta["st_tiles"]):
                aggregate(s, ts, (cc1[2], cc1[3]), rb1_sb, l1_out)

    nc.compile()
    return nc


# ---------------------------------------------------------------------------
# entry point
# ---------------------------------------------------------------------------

def _prepare(x, knn_edge_index, W_l0, b_l0, W_r0, W_l1, b_l1, W_r1,
             NC=8, tiles_per_st=2):
    x = np.asarray(x, np.float32)
    e = np.asarray(knn_edge_index)
    in_dim, N = x.shape
    src, dst = e[0].astype(np.int64), e[1].astype(np.int64)
    meta = build_meta(N, NC, dst, src, tiles_per_st)
    npc, NPAD = meta["npc"], meta["NPAD"]
    GC = (in_dim + P - 1) // P
    GPAD = GC * P

    w0l = np.zeros((GPAD, P), np.float32); w0l[:in_dim] = np.asarray(W_l0).T
    w0r = np.zeros((GPAD, P), np.float32); w0r[:in_dim] = np.asarray(W_r0).T
    shared = {
        "W0lT": w0l, "W0rT": w0r,
        "W1lT": np.ascontiguousarray(np.asarray(W_l1, np.float32).T),
        "W1rT": np.ascontiguousarray(np.asarray(W_r1, np.float32).T),
        "b0col": np.asarray(b_l0, np.float32).reshape(P, 1),
        "b1col": np.asarray(b_l1, np.float32).reshape(P, 1),
        "iota": np.broadcast_to(np.arange(P, dtype=np.float32), (P, P)).copy(),
    }
    in_maps = []
    for c in range(NC):
        xp = np.zeros((GPAD, NPAD), np.float32)
        xp[:in_dim, :npc] = x[:, c * npc:(c + 1) * npc]
        m = dict(shared)
        m["x_pad"] = xp
        m["invt"] = np.ascontiguousarray(
            np.broadcast_to(meta["inv"][c], (P, NPAD)))
        m["idx16"] = np.ascontiguousarray(meta["idx_slab"][c])
        m["dstid"] = np.ascontiguousarray(meta["dstid_slab"][c])
        in_maps.append(m)
    return meta, in_dim, in_maps


def run(inputs, NC=8, tiles_per_st=2, trace=False, **run_kwargs):
    meta, in_dim, in_maps = _prepare(**inputs, NC=NC, tiles_per_st=tiles_per_st)
    nc = build_kernel(meta, in_dim, NC)
    res = bass_utils.run_bass_kernel_spmd(
        nc, in_maps, core_ids=list(range(NC)), trace=trace, **run_kwargs)
    npc = meta["npc"]
    out = np.concatenate(
        [res.results[c]["outT"][:, :npc].T for c in range(NC)], axis=0)
    return np.ascontiguousarray(out), res


def kernel(**inputs) -> np.ndarray:
    out, _ = run(inputs)
    return out


# revision 13
# speedup vs baseline: 1.7926x; 1.3481x over previous
"""Trainium2 Bass kernel for nn_CellEncoder (2-layer GraphSAGE, mean aggregation).

Strategy (8 NeuronCores, SPMD, node-partitioned):
  - Core c owns nodes [c*npc, (c+1)*npc).  Aggregation is linear, so the
    dense transform is applied FIRST: z = h @ W_l.T reduces gather width
    from in_dim (1000) to emb (128) floats per edge.
  - Per layer: each core computes z for its own nodes (node-major [n,128]
    rows), contributes two half-slabs to two AllGathers forming
    table_lo/table_hi (each NC*npc/2 rows < 32768 so rows are addressable
    by int16 dma_gather indices).
  - Edges are grouped by dst tile (128 dsts); each tile's edges are packed
    into chunks of 128 slots (lo-table chunks then hi-table chunks).
    dma_gather (multi-packet, round-robin over SWDGE queues 1-3) pulls the
    slot rows into SBUF; host-precomputed one-hot matrices
    S[e,d] = (dst(e)==d), stored bf16 and cast-loaded to f32 on queue 0,
    drive the PE accumulation aggT[f,d] += G_chunk.T @ S_chunk in PSUM.
  - Epilogue is feature-major (transposed): x = aggT*inv + rT + b;
    h = elu(x) = (max(x,0) + exp(min(x,0)) - 1), all via tensor_tensor
    with [128,1]-broadcast constants (tensor_scalar measured 3-10x slower
    on HW).  Feature-major h feeds the next layer's matmuls directly.
  - Output written feature-major [128, NPAD]; host transposes/trims.

kernel(**inputs) takes FULL inputs, shards internally, runs one NEFF on
cores 0-7 via bass_utils.run_bass_kernel_spmd, returns the full output.
"""
import os
import sys

import numpy as np

for _p in ("/opt/trn_rl_repo", "/root/.axon_site/_ro/trn_rl_repo"):
    if os.path.isdir(_p) and _p not in sys.path:
        sys.path.append(_p)

import ml_dtypes

import concourse.bass as bass
import concourse.bacc as bacc
import concourse.mybir as mybir
import concourse.tile as tile
from concourse import bass_utils

P = 128
F32 = mybir.dt.float32
BF16 = mybir.dt.bfloat16
AF = mybir.ActivationFunctionType
ALU = mybir.AluOpType

# SWDGE descriptor-ring sizing: ring holds scratch//16 descriptors; one
# dma_gather must fit in its queue's ring.  Measured on HW: >1024 idxs needs
# single_packet=False; 2816-idx gathers at scratch=49152/4 queues run at
# ~2.9ns/idx.  Keep gathers <= GMAX chunks so they fit the ring.
SCRATCH = 32768
GMAX = 16  # chunks (2048 idxs) per dma_gather


def build_meta(N, NC, dst, src, tiles_per_st):
    """Static chunk structure (shared across cores; max-over-core sizes) and
    per-core gather-index / one-hot slabs."""
    npc = N // NC
    half = npc // 2
    TPC = (npc + P - 1) // P
    NPAD = TPC * P
    NST = (TPC + tiles_per_st - 1) // tiles_per_st

    c = dst // npc
    d = (dst - c * npc).astype(np.int64)
    t = d // P
    did = d % P
    sc = src // npc
    sp = src - sc * npc
    tb = (sp >= half).astype(np.int64)
    row = sc * half + np.where(tb == 0, sp, sp - half)
    assert row.max() < 32768

    nlohi = np.zeros((NC, TPC, 2), np.int64)
    np.add.at(nlohi, (c, t, tb), 1)
    KL = np.maximum(1, (nlohi[:, :, 0].max(axis=0) + P - 1) // P)
    KH = ((nlohi[:, :, 1].max(axis=0) + P - 1) // P).astype(np.int64)

    Ktot = KL + KH
    chunk_base = np.concatenate([[0], np.cumsum(Ktot)])
    NCHUNK = int(chunk_base[-1])

    st_tiles = [list(range(s * tiles_per_st, min((s + 1) * tiles_per_st, TPC)))
                for s in range(NST)]
    GL = [int(sum(KL[tt] for tt in ts)) for ts in st_tiles]
    GH = [int(sum(KH[tt] for tt in ts)) for ts in st_tiles]

    idx_off_lo, idx_off_hi = [], []
    off = 0
    for s in range(NST):
        idx_off_lo.append(off); off += GL[s] * P // 16
        idx_off_hi.append(off); off += GH[s] * P // 16
    NIDX16 = off

    idx_slab = np.zeros((NC, P, NIDX16), np.int16)
    dstid_slab = np.full((NC, P, NCHUNK), -1.0, np.float32)
    cnt = np.zeros((NC, NPAD), np.int64)

    order = np.lexsort((tb, t, c))
    co, to, tbo = c[order], t[order], tb[order]
    rowo, dido, do_ = row[order], did[order], d[order]
    np.add.at(cnt, (co, do_), 1)

    key = (co * TPC + to) * 2 + tbo
    bounds = np.concatenate([[0], np.nonzero(np.diff(key))[0] + 1, [len(key)]])
    gval_lo = [np.zeros((NC, GL[s] * P), np.int16) for s in range(NST)]
    gval_hi = [np.zeros((NC, GH[s] * P), np.int16) for s in range(NST)]

    lo_base = np.zeros(TPC, np.int64)
    hi_base = np.zeros(TPC, np.int64)
    for s, ts in enumerate(st_tiles):
        accl = acch = 0
        for tt in ts:
            lo_base[tt] = accl; accl += KL[tt] * P
            hi_base[tt] = acch; acch += KH[tt] * P

    for bi in range(len(bounds) - 1):
        lo_, hi_ = int(bounds[bi]), int(bounds[bi + 1])
        if lo_ == hi_:
            continue
        cc, tt, bb = int(co[lo_]), int(to[lo_]), int(tbo[lo_])
        n = hi_ - lo_
        s = tt // tiles_per_st
        if bb == 0:
            base = int(lo_base[tt])
            gval_lo[s][cc, base:base + n] = rowo[lo_:hi_]
            ch0 = int(chunk_base[tt])
        else:
            base = int(hi_base[tt])
            gval_hi[s][cc, base:base + n] = rowo[lo_:hi_]
            ch0 = int(chunk_base[tt]) + int(KL[tt])
        # base is a multiple of P: slot partition (base+i)%P == i%P and
        # gather block base//P + i//P lines up with tile chunk ch0 + i//P.
        local = np.arange(n)
        dstid_slab[cc, local % P, ch0 + local // P] = dido[lo_:hi_]

    for s in range(NST):
        for cc in range(NC):
            for vals, o in ((gval_lo[s][cc], idx_off_lo[s]),
                            (gval_hi[s][cc], idx_off_hi[s])):
                n = len(vals)
                if n == 0:
                    continue
                w = vals.reshape(n // 16, 16).T
                idx_slab[cc, :, o:o + n // 16] = np.tile(w, (8, 1))

    # one-hot S slab, bf16: s_slab[c, e, cg*128 + d] = (dstid[c,e,cg] == d)
    oneh = (dstid_slab[:, :, :, None] ==
            np.arange(P, dtype=np.float32)[None, None, None, :])
    s_slab = np.ascontiguousarray(
        oneh.astype(ml_dtypes.bfloat16).reshape(NC, P, NCHUNK * P))

    inv = (1.0 / np.maximum(cnt, 1)).astype(np.float32)

    return dict(
        npc=npc, half=half, TPC=TPC, NPAD=NPAD, NST=NST, st_tiles=st_tiles,
        KL=[int(v) for v in KL], KH=[int(v) for v in KH],
        chunk_base=[int(v) for v in chunk_base], NCHUNK=NCHUNK,
        GL=GL, GH=GH, idx_off_lo=idx_off_lo, idx_off_hi=idx_off_hi,
        NIDX16=NIDX16, idx_slab=idx_slab, s_slab=s_slab, inv=inv,
    )


# ---------------------------------------------------------------------------
# device kernel builder
# ---------------------------------------------------------------------------

def build_kernel(meta, in_dim, NC):
    npc, half = meta["npc"], meta["half"]
    TPC, NPAD, NST = meta["TPC"], meta["NPAD"], meta["NST"]
    NCHUNK, NIDX16 = meta["NCHUNK"], meta["NIDX16"]
    KL, KH, chunk_base = meta["KL"], meta["KH"], meta["chunk_base"]
    GC = (in_dim + P - 1) // P
    GPAD = GC * P
    gq = [0]  # gather queue round-robin over 1..3 (0 carries S cast-loads)

    nc = bacc.Bacc("TRN2", target_bir_lowering=False, debug=False,
                   enable_asserts=False, num_devices=NC,
                   dynamic_dma_scratch_size=SCRATCH, num_swdge_queues=4)

    x_d = nc.dram_tensor("x_pad", [GPAD, NPAD], F32, kind="ExternalInput").ap()
    w0l_d = nc.dram_tensor("W0lT", [GPAD, P], F32, kind="ExternalInput").ap()
    w0r_d = nc.dram_tensor("W0rT", [GPAD, P], F32, kind="ExternalInput").ap()
    w1l_d = nc.dram_tensor("W1lT", [P, P], F32, kind="ExternalInput").ap()
    w1r_d = nc.dram_tensor("W1rT", [P, P], F32, kind="ExternalInput").ap()
    b0_d = nc.dram_tensor("b0col", [P, 1], F32, kind="ExternalInput").ap()
    b1_d = nc.dram_tensor("b1col", [P, 1], F32, kind="ExternalInput").ap()
    inv_d = nc.dram_tensor("invt", [P, NPAD], F32, kind="ExternalInput").ap()
    idx_d = nc.dram_tensor("idx16", [P, NIDX16], mybir.dt.int16,
                           kind="ExternalInput").ap()
    s_d = nc.dram_tensor("soneh", [P, NCHUNK * P], BF16,
                         kind="ExternalInput").ap()
    out_d = nc.dram_tensor("outT", [P, NPAD], F32, kind="ExternalOutput").ap()

    with tile.TileContext(nc, num_cores=NC) as tc:
        with (
            tc.tile_pool(name="const", bufs=1) as cpool,
            tc.tile_pool(name="slab", bufs=1) as slab,
            tc.tile_pool(name="xp", bufs=2) as xpool,
            tc.tile_pool(name="gat", bufs=2) as gpool,
            tc.tile_pool(name="sp", bufs=2) as spool,
            tc.tile_pool(name="ip", bufs=2) as ipool,
            tc.tile_pool(name="xip", bufs=2) as xipool,
            tc.tile_pool(name="zp", bufs=3) as zpool,
            tc.tile_pool(name="ep", bufs=2) as epool,
            tc.tile_pool(name="pz", bufs=3, space="PSUM") as pz,
            tc.tile_pool(name="pr", bufs=2, space="PSUM") as pr,
            tc.tile_pool(name="pa", bufs=2, space="PSUM") as pa,
            tc.tile_pool(name="dram", bufs=1, space="DRAM") as dram,
        ):
            # ---- constants ----
            w0l_sb = cpool.tile([P, GC * P], F32)
            w0r_sb = cpool.tile([P, GC * P], F32)
            for gc in range(GC):
                nc.sync.dma_start(out=w0l_sb[:, gc * P:(gc + 1) * P],
                                  in_=w0l_d[gc * P:(gc + 1) * P, :])
                nc.sync.dma_start(out=w0r_sb[:, gc * P:(gc + 1) * P],
                                  in_=w0r_d[gc * P:(gc + 1) * P, :])
            w1l_sb = cpool.tile([P, P], F32)
            nc.sync.dma_start(out=w1l_sb[:], in_=w1l_d[:])
            w1r_sb = cpool.tile([P, P], F32)
            nc.sync.dma_start(out=w1r_sb[:], in_=w1r_d[:])
            b0_sb = cpool.tile([P, 1], F32)
            nc.sync.dma_start(out=b0_sb[:], in_=b0_d[:])
            b1_sb = cpool.tile([P, 1], F32)
            nc.sync.dma_start(out=b1_sb[:], in_=b1_d[:])
            zero_sb = cpool.tile([P, 1], F32)
            nc.vector.memset(zero_sb[:], 0.0)
            mone_sb = cpool.tile([P, 1], F32)
            nc.vector.memset(mone_sb[:], -1.0)
            IDXW = max(meta["GL"][s] + meta["GH"][s] for s in range(NST)) * 8

            rb0_sb = slab.tile([P, NPAD], F32)
            rb1_sb = slab.tile([P, NPAD], F32)

            # ---- collective buffers ----
            def cc_pair(nm):
                i_lo = dram.tile([half, P], F32, name=f"cci_lo{nm}")
                i_hi = dram.tile([half, P], F32, name=f"cci_hi{nm}")
                o_lo = dram.tile([NC * half, P], F32, addr_space="Shared",
                                 name=f"cco_lo{nm}")
                o_hi = dram.tile([NC * half, P], F32, addr_space="Shared",
                                 name=f"cco_hi{nm}")
                return i_lo, i_hi, o_lo, o_hi

            cc0 = cc_pair("0")
            cc1 = cc_pair("1")
            rg = [list(range(NC))]

            def z_to_cc(z_sb, tt, cc):
                r0, r1 = tt * P, min(tt * P + P, npc)
                for lo_s, hi_s, tgt, base in (
                        (r0, min(r1, half), cc[0], 0),
                        (max(r0, half), r1, cc[1], half)):
                    if hi_s > lo_s:
                        nc.sync.dma_start(
                            out=tgt[lo_s - base:hi_s - base, :],
                            in_=z_sb[lo_s - r0:hi_s - r0, :])

            def bcast(col_ap):
                return col_ap.to_broadcast([P, P])

            # ---- phase A: z0 (node-major) + rb0T (feature-major) ----
            for s, ts in enumerate(meta["st_tiles"]):
                w = len(ts) * P
                c0 = ts[0] * P
                xg = xpool.tile([P, GC * w], F32, tag="xg",
                                padded_shape=[P, GC * 2 * P])
                nc.sync.dma_start(
                    out=xg[:].rearrange("p (gc j) -> p gc j", gc=GC),
                    in_=x_d[:, c0:c0 + w].rearrange("(gc p) j -> p gc j", p=P))
                r0ps = pr.tile([P, w], F32, tag="rps", padded_shape=[P, 2 * P])
                for gc in range(GC):
                    nc.tensor.matmul(out=r0ps[:],
                                     lhsT=w0r_sb[:, gc * P:(gc + 1) * P],
                                     rhs=xg[:, gc * w:(gc + 1) * w],
                                     start=(gc == 0), stop=(gc == GC - 1))
                nc.vector.tensor_tensor(out=rb0_sb[:, c0:c0 + w], in0=r0ps[:],
                                        in1=b0_sb[:, :1].to_broadcast([P, w]),
                                        op=ALU.add)
                for ti, tt in enumerate(ts):
                    z0ps = pz.tile([P, P], F32, tag="zps")
                    for gc in range(GC):
                        nc.tensor.matmul(
                            out=z0ps[:],
                            lhsT=xg[:, gc * w + ti * P:gc * w + (ti + 1) * P],
                            rhs=w0l_sb[:, gc * P:(gc + 1) * P],
                            start=(gc == 0), stop=(gc == GC - 1))
                    z0sb = zpool.tile([P, P], F32, tag="zsb")
                    nc.vector.tensor_copy(out=z0sb[:], in_=z0ps[:])
                    z_to_cc(z0sb, tt, cc0)

            nc.gpsimd.collective_compute(
                "AllGather", ALU.bypass, replica_groups=rg,
                ins=[cc0[0][:].opt()], outs=[cc0[2][:].opt()])
            nc.gpsimd.collective_compute(
                "AllGather", ALU.bypass, replica_groups=rg,
                ins=[cc0[1][:].opt()], outs=[cc0[3][:].opt()])

            def gather_split(table, nch, idx_sb, idx_off, tag):
                """One or more dma_gathers (<= GMAX chunks each) into one
                SBUF tile [P, nch*P]."""
                if nch == 0:
                    return None
                g = gpool.tile([P, nch * P], F32, tag=tag,
                               padded_shape=[P, (max(meta["GL"] + meta["GH"])) * P])
                done = 0
                while done < nch:
                    n = min(GMAX, nch - done)
                    gq[0] = gq[0] % 3 + 1
                    nc.gpsimd.dma_gather(
                        out_ap=g[:, done * P:(done + n) * P]
                        .rearrange("p (k e) -> p k e", e=P),
                        in_ap=table[:],
                        idxs_ap=idx_sb[:, idx_off + done * 8:
                                       idx_off + (done + n) * 8],
                        num_idxs=n * P, num_idxs_reg=n * P, elem_size=P,
                        single_packet=(n * P <= 1024), queue_num=gq[0])
                    done += n
                return g

            # ---- aggregation + epilogue (shared for both layers) ----
            def aggregate(s, ts, tables, rb_slab, out_cb):
                w = len(ts) * P
                c0 = ts[0] * P
                GLs, GHs = meta["GL"][s], meta["GH"][s]
                nch_st = GLs + GHs
                idxt = xipool.tile([P, nch_st * 8], mybir.dt.int16, tag="idxt",
                                   padded_shape=[P, IDXW])
                o_lo = meta["idx_off_lo"][s]
                nc.sync.dma_start(out=idxt[:],
                                  in_=idx_d[:, o_lo:o_lo + nch_st * 8])
                glo = gather_split(tables[0], GLs, idxt, 0, "glo")
                ghi = gather_split(tables[1], GHs, idxt, GLs * 8, "ghi")
                # one-hot S slab for this supertile (bf16 -> f32 SWDGE cast)
                sch0 = chunk_base[ts[0]]
                s_sb = spool.tile([P, nch_st * P], F32, tag="ssb",
                                  padded_shape=[P, (max(meta["GL"] + meta["GH"]) * 2) * P])
                nc.gpsimd.dma_start(
                    out=s_sb[:],
                    in_=s_d[:, sch0 * P:(sch0 + nch_st) * P])
                aggps = pa.tile([P, w], F32, tag="aggps", padded_shape=[P, 2 * P])
                lo_blk = hi_blk = 0
                soff = 0
                for ti, tt in enumerate(ts):
                    nch = KL[tt] + KH[tt]
                    for j in range(nch):
                        if j < KL[tt]:
                            g_ap = glo[:, (lo_blk + j) * P:(lo_blk + j + 1) * P]
                        else:
                            jj = j - KL[tt]
                            g_ap = ghi[:, (hi_blk + jj) * P:(hi_blk + jj + 1) * P]
                        nc.tensor.matmul(out=aggps[:, ti * P:(ti + 1) * P],
                                         lhsT=g_ap,
                                         rhs=s_sb[:, (soff + j) * P:
                                                  (soff + j + 1) * P],
                                         start=(j == 0), stop=(j == nch - 1))
                    lo_blk += KL[tt]
                    hi_blk += KH[tt]
                    soff += nch
                # epilogue: x = aggT*inv + rb ; h = max(x,0) + exp(min(x,0)) - 1
                invt = ipool.tile([P, w], F32, tag="invt",
                                  padded_shape=[P, 2 * P])
                nc.sync.dma_start(out=invt[:], in_=inv_d[:, c0:c0 + w])
                x2 = epool.tile([P, w], F32, tag="x2", padded_shape=[P, 2 * P])
                nc.vector.tensor_tensor(out=x2[:], in0=aggps[:],
                                        in1=invt[:], op=ALU.mult)
                x3 = epool.tile([P, w], F32, tag="x3", padded_shape=[P, 2 * P])
                nc.vector.tensor_tensor(out=x3[:], in0=x2[:],
                                        in1=rb_slab[:, c0:c0 + w], op=ALU.add)
                xm = epool.tile([P, w], F32, tag="xm", padded_shape=[P, 2 * P])
                nc.vector.tensor_tensor(out=xm[:], in0=x3[:],
                                        in1=zero_sb[:, :1].to_broadcast([P, w]),
                                        op=ALU.max)
                xc = epool.tile([P, w], F32, tag="xc", padded_shape=[P, 2 * P])
                nc.vector.tensor_tensor(out=xc[:], in0=x3[:],
                                        in1=zero_sb[:, :1].to_broadcast([P, w]),
                                        op=ALU.min)
                xe = epool.tile([P, w], F32, tag="xe", padded_shape=[P, 2 * P])
                nc.scalar.activation(out=xe[:], in_=xc[:], func=AF.Exp)
                xs = epool.tile([P, w], F32, tag="xs", padded_shape=[P, 2 * P])
                nc.vector.tensor_tensor(out=xs[:], in0=xm[:], in1=xe[:],
                                        op=ALU.add)
                h = epool.tile([P, w], F32, tag="h", padded_shape=[P, 2 * P])
                nc.vector.tensor_tensor(out=h[:], in0=xs[:],
                                        in1=mone_sb[:, :1].to_broadcast([P, w]),
                                        op=ALU.add)
                out_cb(s, ts, w, c0, h)

            # ---- phase B+C: layer-0 aggregate -> h1T -> z1/rb1T ----
            def l0_out(s, ts, w, c0, h):
                for ti, tt in enumerate(ts):
                    z1ps = pz.tile([P, P], F32, tag="zps")
                    nc.tensor.matmul(out=z1ps[:],
                                     lhsT=h[:, ti * P:(ti + 1) * P],
                                     rhs=w1l_sb[:], start=True, stop=True)
                    z1sb = zpool.tile([P, P], F32, tag="zsb")
                    nc.vector.tensor_copy(out=z1sb[:], in_=z1ps[:])
                    z_to_cc(z1sb, tt, cc1)
                r1ps = pr.tile([P, w], F32, tag="rps", padded_shape=[P, 2 * P])
                nc.tensor.matmul(out=r1ps[:], lhsT=w1r_sb[:], rhs=h[:],
                                 start=True, stop=True)
                nc.vector.tensor_tensor(out=rb1_sb[:, c0:c0 + w], in0=r1ps[:],
                                        in1=b1_sb[:, :1].to_broadcast([P, w]),
                                        op=ALU.add)

            for s, ts in enumerate(meta["st_tiles"]):
                aggregate(s, ts, (cc0[2], cc0[3]), rb0_sb, l0_out)

            nc.gpsimd.collective_compute(
                "AllGather", ALU.bypass, replica_groups=rg,
                ins=[cc1[0][:].opt()], outs=[cc1[2][:].opt()])
            nc.gpsimd.collective_compute(
                "AllGather", ALU.bypass, replica_groups=rg,
                ins=[cc1[1][:].opt()], outs=[cc1[3][:].opt()])

            # ---- phase D: layer-1 aggregate -> output ----
            def l1_out(s, ts, w, c0, h):
                nc.sync.dma_start(out=out_d[:, c0:c0 + w], in_=h[:])

            for s, ts in enumerate(meta["st_tiles"]):
                aggregate(s, ts, (cc1[2], cc1[3]), rb1_sb, l1_out)

    nc.compile()
    return nc


# ---------------------------------------------------------------------------
# entry point
# ---------------------------------------------------------------------------

def _prepare(x, knn_edge_index, W_l0, b_l0, W_r0, W_l1, b_l1, W_r1,
             NC=8, tiles_per_st=2):
    x = np.asarray(x, np.float32)
    e = np.asarray(knn_edge_index)
    in_dim, N = x.shape
    src, dst = e[0].astype(np.int64), e[1].astype(np.int64)
    meta = build_meta(N, NC, dst, src, tiles_per_st)
    npc, NPAD = meta["npc"], meta["NPAD"]
    GC = (in_dim + P - 1) // P
    GPAD = GC * P

    w0l = np.zeros((GPAD, P), np.float32); w0l[:in_dim] = np.asarray(W_l0).T
    w0r = np.zeros((GPAD, P), np.float32); w0r[:in_dim] = np.asarray(W_r0).T
    shared = {
        "W0lT": w0l, "W0rT": w0r,
        "W1lT": np.ascontiguousarray(np.asarray(W_l1, np.float32).T),
        "W1rT": np.ascontiguousarray(np.asarray(W_r1, np.float32).T),
        "b0col": np.asarray(b_l0, np.float32).reshape(P, 1),
        "b1col": np.asarray(b_l1, np.float32).reshape(P, 1),
    }
    in_maps = []
    for c in range(NC):
        xp = np.zeros((GPAD, NPAD), np.float32)
        xp[:in_dim, :npc] = x[:, c * npc:(c + 1) * npc]
        m = dict(shared)
        m["x_pad"] = xp
        m["invt"] = np.ascontiguousarray(
            np.broadcast_to(meta["inv"][c], (P, NPAD)))
        m["idx16"] = np.ascontiguousarray(meta["idx_slab"][c])
        m["soneh"] = meta["s_slab"][c]
        in_maps.append(m)
    return meta, in_dim, in_maps


def run(inputs, NC=8, tiles_per_st=2, trace=False, **run_kwargs):
    meta, in_dim, in_maps = _prepare(**inputs, NC=NC, tiles_per_st=tiles_per_st)
    nc = build_kernel(meta, in_dim, NC)
    res = bass_utils.run_bass_kernel_spmd(
        nc, in_maps, core_ids=list(range(NC)), trace=trace, **run_kwargs)
    npc = meta["npc"]
    out = np.concatenate(
        [res.results[c]["outT"][:, :npc].T for c in range(NC)], axis=0)
    return np.ascontiguousarray(out), res


def kernel(**inputs) -> np.ndarray:
    out, _ = run(inputs)
    return out
